# revision 2
# baseline (speedup 1.0000x reference)
"""Trainium2 Bass kernel v2 for nn_Att_61881888801149 (sparse_attention).

Math per batch b (softmax over the QUERY axis l):
    out[l, c] = sum_m E[l, m] * v[m, c] / colsum[m],  E = exp(S - 11.5)
    S = q (k^T + pos)

Sharding: 8 cores = 4 batches x 2 key-halves (M = 2048 keys per core).
Host rotates xT columns per-core so each core's m-half is always columns
0:2048; outputs are un-rotated and pair-summed on the host.

v2 schedule (vs v1, 114.4us -> target ~96us): the v1 timeline was
  [ACT-bound exp stream 4..88us, PE ~45% idle] then [PE-only phase-2
  tail 88..110us]. Here phase-2 runs INSIDE the exp stream:
  - out accumulation is split into NG m-slabs; each slab's partial out
    tile goes PSUM -> DRAM directly (no on-chip add chain); the host sums
    the slabs.  This keeps DVE off the critical path entirely and lets
    slabs be small (fine-grained PE feed as each E[mb] lands).
  - PSUM: 2x[128,1024] ST chunks (ACT-paced) + 4x[128,512] rotating
    slots shared by projection psums (early) and slab accumulators.
  - q/v bias matmuls removed: folded into the DVE psum->SBUF copies
    (bq rides in pos's last column; bv is pre-broadcast in xw rows).
  - colsum reduce + reciprocal on DVE; vbw scaling on the idle Pool.
"""

import sys

for _p in ("/opt/trn_rl_repo", "/root/.axon_site/_ro/trn_rl_repo"):
    if _p not in sys.path:
        sys.path.append(_p)

import numpy as np

B, L, C, Cr = 4, 4096, 256, 32
MH = L // 2
NCORES = 8

_CACHE = {}

# phase-2 m-slabs: per out-tile staggered segment boundaries so chain
# ready-times spread uniformly across the exp stream (NMB=16 m-blocks).
# 4 segments per tile -> outT carries 4 partial slabs summed on the host.
NG = 4


def tile_segs(t):
    s1 = 2 + t // 4                 # 2..5
    s2 = 8 + (t * 5) // 16          # 8..12
    s3 = 14 + (t * 2) // 16         # 14..15
    return [0, s1, s2, s3, 16]


def build_nc(L=L, C=C, Cr=Cr, M=MH):
    import concourse.bass as bass
    import concourse.tile as tile
    from concourse import mybir
    from concourse.tile_rust import add_dep_helper

    FP32 = mybir.dt.float32
    FP16 = mybir.dt.float16
    Exp = mybir.ActivationFunctionType.Exp
    EXP_OFF = 11.5

    assert C == 256 and Cr == 32
    NMB = M // 128          # 16 m-blocks per core
    NLG = L // 512          # 8 l-groups
    SG = 1024               # ACT exp chunk width
    NSG = L // SG           # 4 stats groups per m-block
    NST = NSG + 1           # +1: mb0's first chunk is split in two
    NCH = C // 128          # 2 output channel halves
    NT = NLG * NCH          # 16 out tiles per slab

    # xw (fp16) columns: wq0 0:32 | wq1 32:64 | wk0 64:96 | wk1 96:128 |
    # wv0 128:384 | wv1 384:640 | bv broadcast (all rows) 640:896
    nc = bass.Bass()
    xt_d = nc.dram_tensor("xt", [128, 2 * L], FP16, kind="ExternalInput")
    xw_d = nc.dram_tensor("xw", [128, 904], FP16, kind="ExternalInput")
    pos_d = nc.dram_tensor("pos", [Cr, M], FP32, kind="ExternalInput")
    bq_d = nc.dram_tensor("bq", [Cr, 1], FP32, kind="ExternalInput")
    outT_d = nc.dram_tensor("outT", [C, NG, L], FP16, kind="ExternalOutput")

    with tile.TileContext(nc) as tc:
        with (
            tc.tile_pool(name="persist", bufs=1) as persist,
            tc.tile_pool(name="psum", bufs=1, space="PSUM") as psum,
        ):
            qT = persist.tile([Cr, L], FP16)
            kpT = persist.tile([Cr, M], FP16)
            vb = persist.tile([128, NMB, C], FP16)
            vbw = persist.tile([128, NMB, C], FP16)
            stats = persist.tile([128, NMB, NST], FP32)
            nc.vector.memset(stats[:, :, NSG:], 0.0)
            colsum = persist.tile([128, NMB], FP32)
            wrec = persist.tile([128, NMB], FP32)
            expoff = persist.tile([128, 1], FP32)
            nc.vector.memset(expoff[:], -EXP_OFF)
            # carrier tile: hoists PSUM-slot WAR waits onto a PE Ldweights
            # so the slot-opening Matmult keeps 1 sem wait (walrus budget)
            wdum = persist.tile([1, 1], FP16)
            nc.vector.memset(wdum[:], 0.0)
            # warm the ACT exp table before the stream
            exw = persist.tile([1, 1], FP32)
            nc.scalar.activation(exw[:], expoff[0:1, 0:1], Exp)

            def carrier(dep):
                if dep is None:
                    return None
                c = nc.tensor.ldweights(wdum[:])
                add_dep_helper(c.ins, dep.ins, sync=True,
                               reason="psum slot WAR carrier")
                return c

            def anchor(mm, c):
                if c is not None:
                    add_dep_helper(mm.ins, c.ins, sync=False,
                                   reason="carrier anchor")
                return mm

            with tc.tile_pool(name="epool", bufs=1) as epool:
                E = epool.tile([128, NMB, L], FP16)

                with tc.tile_pool(name="prolog", bufs=1) as pp:
                    bqc = pp.tile([Cr, 1], FP32)
                    xw = pp.tile([128, 904], FP16)
                    pos = pp.tile([Cr, M], FP32)
                    xt = pp.tile([128, 2, L], FP16)
                    # sync queue: xt half 0 (first 512 lands first).
                    # gpsimd queue: weights/bias/pos interleaved with xt
                    # half 1 so everything the first chunks need lands early.
                    # No DMAs on the ACT queue (they'd block the exp stream).
                    spans = [(0, 256), (256, 512), (512, 1024)] + [
                        (j * 1024, (j + 1) * 1024) for j in range(1, L // 1024)]
                    for a, b in spans:
                        nc.sync.dma_start(xt[:, 0, a:b],
                                          xt_d[:, a:b])
                    nc.gpsimd.dma_start(xw[:], xw_d[:])
                    nc.gpsimd.dma_start(xt[:, 1, 0:512], xt_d[:, L:L + 512])
                    nc.gpsimd.dma_start(bqc[:], bq_d[:])
                    nc.gpsimd.dma_start(xt[:, 1, 512:1024],
                                        xt_d[:, L + 512:L + 1024])
                    nc.gpsimd.dma_start(pos[:, 0:512], pos_d[:, 0:512])
                    for j in range(1, L // 1024):
                        nc.gpsimd.dma_start(
                            xt[:, 1, j * 1024:(j + 1) * 1024],
                            xt_d[:, L + j * 1024:L + (j + 1) * 1024])
                    nc.gpsimd.dma_start(pos[:, 512:M], pos_d[:, 512:M])

                    wq0, wq1 = xw[:, 0:32], xw[:, 32:64]
                    wk0, wk1 = xw[:, 64:96], xw[:, 96:128]
                    wv0, wv1 = xw[:, 128:384], xw[:, 384:640]
                    bvb = xw[:, 640:896]

                    # rotating 4-slot PSUM pool: proj psums early, slab
                    # accumulators after
                    hist_oa = [None, None, None, None]
                    hist_st = [None, None]
                    kidx = [0]

                    def oa_tile(name):
                        cr_ = carrier(hist_oa[kidx[0] % 4])
                        t = psum.tile([128, 512], FP32, tag="oa", bufs=4,
                                      name=name)
                        return t, cr_

                    def oa_done(reader):
                        hist_oa[kidx[0] % 4] = reader
                        kidx[0] += 1

                    qdone, kdone, vdone = set(), set(), set()

                    def need_q(j):
                        if j in qdone:
                            return
                        qdone.add(j)
                        sl = slice(j * 512, (j + 1) * 512)
                        psq_t, cr_ = oa_tile(f"psq_{j}")
                        psq = psq_t[0:Cr, 0:512]
                        anchor(nc.tensor.matmul(psq[:], wq0, xt[:, 0, sl],
                                                start=True, stop=False), cr_)
                        nc.tensor.matmul(psq[:], wq1, xt[:, 1, sl],
                                         start=False, stop=True)
                        oa_done(nc.vector.tensor_scalar_add(
                            qT[:, sl], psq[:], bqc))

                    def need_k(j):
                        if j in kdone:
                            return
                        kdone.add(j)
                        sl = slice(j * 512, (j + 1) * 512)
                        psk_t, cr_ = oa_tile(f"psk_{j}")
                        psk = psk_t[0:Cr, 0:512]
                        anchor(nc.tensor.matmul(psk[:], wk0, xt[:, 0, sl],
                                                start=True, stop=False), cr_)
                        nc.tensor.matmul(psk[:], wk1, xt[:, 1, sl],
                                         start=False, stop=True)
                        oa_done(nc.vector.tensor_add(
                            kpT[:, sl], psk[:], pos[:, sl]))

                    def need_v(mb):
                        if mb in vdone:
                            return
                        vdone.add(mb)
                        msl = slice(mb * 128, (mb + 1) * 128)
                        psv_t, cr_ = oa_tile(f"psv_{mb}")
                        psv = psv_t[:, 0:C]
                        anchor(nc.tensor.matmul(psv[:], xt[:, 0, msl], wv0,
                                                start=True, stop=False), cr_)
                        nc.tensor.matmul(psv[:], xt[:, 1, msl], wv1,
                                         start=False, stop=True)
                        oa_done(nc.vector.tensor_add(vb[:, mb, :], psv[:],
                                                     bvb))

                    def stats_tail(mb):
                        nst = NST if mb == 0 else NSG
                        nc.vector.reduce_sum(colsum[:, mb:mb + 1],
                                             stats[:, mb, 0:nst],
                                             axis=mybir.AxisListType.X)
                        nc.vector.reciprocal(wrec[:, mb:mb + 1],
                                             colsum[:, mb:mb + 1])
                        nc.vector.tensor_scalar_mul(
                            vbw[:, mb, :], vb[:, mb, :], wrec[:, mb:mb + 1])

                    stc = [0]

                    def st_chunk(mb, a, b, si_):
                        kp_sl = kpT[:, mb * 128:(mb + 1) * 128]
                        idx = stc[0]
                        stc[0] += 1
                        cr_ = carrier(hist_st[idx % 2])
                        ps = psum.tile([128, SG], FP32, tag="st", bufs=2)
                        for j, c0 in enumerate(range(a, b, 512)):
                            mm = nc.tensor.matmul(
                                ps[:, j * 512:j * 512 + 512],
                                kp_sl, qT[:, c0:c0 + 512],
                                start=True, stop=True)
                            if j == 0:
                                anchor(mm, cr_)
                        last_exp = nc.scalar.activation(
                            E[:, mb, a:b], ps[:, 0:b - a], Exp,
                            bias=expoff[:],
                            accum_out=stats[:, mb, si_:si_ + 1])
                        hist_st[idx % 2] = last_exp

                    # ---- phase-2 slab chains ----
                    # chain (si, t): accumulate the si-th mb segment of out
                    # tile t=(lg,ch) in a PSUM slot (depth-first), flush via
                    # an engine copy to fp16 staging, then DMA the partial
                    # slab to DRAM (the host sums the NG slabs).
                    dmaq = [nc.sync, nc.sync, nc.gpsimd]
                    Copy = mybir.ActivationFunctionType.Copy
                    flip = [0]
                    dmn = [0]

                    def chain(si, t):
                        segs = tile_segs(t)
                        m0, m1 = segs[si], segs[si + 1]
                        lg, ch = t // NCH, t % NCH
                        lsl = slice(lg * 512, (lg + 1) * 512)
                        poa, cr_ = oa_tile(f"poa_{si}_{t}")
                        for mb in range(m0, m1):
                            mm = nc.tensor.matmul(
                                poa[:],
                                vbw[:, mb, ch * 128:(ch + 1) * 128],
                                E[:, mb, lsl],
                                start=(mb == m0), stop=(mb == m1 - 1))
                            if mb == m0:
                                anchor(mm, cr_)
                        stg = stage.tile([128, 512], FP16, tag="stg")
                        if si == NG - 1 and flip[0] % 2 == 0:
                            rd = nc.scalar.activation(stg[:], poa[:], Copy)
                        else:
                            rd = nc.vector.tensor_copy(stg[:], poa[:])
                        if si == NG - 1:
                            flip[0] += 1
                        oa_done(rd)
                        if si == NG - 1:
                            dq = [nc.gpsimd, nc.sync][dmn[0] % 2]
                        else:
                            dq = dmaq[dmn[0] % 3]
                        dq.dma_start(
                            outT_d[ch * 128:(ch + 1) * 128, si, lsl],
                            stg[:])
                        dmn[0] += 1

                    # static emission schedule keyed by exp-chunk index:
                    # chain (si,t) is ready once E[m1-1] lands (chunk
                    # 4*m1-1); emit with +2 skew (PE runs ~2 chunks ahead),
                    # capped at one chain-open per chunk slot.
                    todo = []
                    for t in range(NT):
                        segs = tile_segs(t)
                        for si in range(NG):
                            todo.append((4 * segs[si + 1] + 3, si, t))
                    todo.sort()
                    events = {}
                    nfree = 0
                    for rci, si, t in todo:
                        ci = max(rci, nfree)
                        nfree = ci + 1
                        events.setdefault(ci, []).append((si, t))

                    def run_events(ci):
                        for si, t in events.pop(ci, []):
                            chain(si, t)

                    with tc.tile_pool(name="stage", bufs=6) as stage:
                        # all 28 projection psums acquire slots before any
                        # chain opens (slot rotation deadlock avoidance)
                        need_q(0)
                        need_q(1)
                        need_k(0)
                        cic = [0]

                        def after_chunk(mb, g):
                            if mb == 0:
                                need_v(g)
                                if g == 3:
                                    need_k(1)
                            elif mb in (1, 2, 3):
                                need_v(4 * mb + g)
                                if g == 3 and mb < 3:
                                    need_k(mb + 1)
                            if g == 0 and mb > 0:
                                stats_tail(mb - 1)
                            run_events(cic[0])
                            cic[0] += 1

                        # mb0: five chunks (512,512,1024x3) so the exp
                        # stream starts as soon as q g0 alone is ready
                        mb0_spans = [(0, 512, 0), (512, 1024, 4),
                                     (1024, 2048, 1), (2048, 3072, 2),
                                     (3072, 4096, 3)]
                        qpre = {0: (2, 3), 1: (4, 5), 2: (6, 7)}
                        for i, (a, b, si_) in enumerate(mb0_spans):
                            st_chunk(0, a, b, si_)
                            for qg in qpre.get(i, ()):
                                need_q(qg)
                            after_chunk(0, min(i, 3))
                        for mb in range(1, NMB):
                            for g in range(NSG):
                                st_chunk(mb, g * SG, (g + 1) * SG, g)
                                after_chunk(mb, g)
                        stats_tail(NMB - 1)
                        for ci in sorted(events):
                            for si, t in events[ci]:
                                chain(si, t)

    return nc


def _fixup_waits(nc):
    """Hoist excess semaphore waits (>1 per TPB instruction) into
    standalone EventSemaphore instructions on the same engine."""
    from concourse import mybir

    budget_by_type = {}
    n = 0
    for fn in nc.m.functions:
        for blk in fn.blocks:
            insts = blk.instructions
            i = 0
            while i < len(insts):
                inst = insts[i]
                si = getattr(inst, "sync_info", None)
                if si is None:
                    i += 1
                    continue
                waits = list(si.on_wait)
                budget = budget_by_type.get(type(inst).__name__, 1)
                if len(waits) <= budget:
                    i += 1
                    continue
                extra, keep = waits[:-budget], waits[-budget:]
                for w in extra:
                    es = mybir.InstEventSemaphore(
                        name=f"es_waitfix_{n}", ins=[], outs=[])
                    n += 1
                    es.engine = inst.engine
                    es.sync_info = mybir.SyncInfo(on_wait=[w], on_update=[])
                    insts.insert(i, es)
                    i += 1
                inst.sync_info = mybir.SyncInfo(
                    on_wait=keep, on_update=list(si.on_update))
                i += 1


def _build_and_fix(**kw):
    nc = build_nc(**kw)
    _fixup_waits(nc)
    return nc


def _get_nc(key, **kw):
    if key not in _CACHE:
        _CACHE[key] = _build_and_fix(**kw)
    return _CACHE[key]


def _prep_core_inputs(x, rel_h, rel_w, Wq, bq, Wk, bk, Wv, bv):
    x = np.asarray(x, dtype=np.float32)
    posf = (np.asarray(rel_h, np.float32) + np.asarray(rel_w, np.float32))
    posf = posf.reshape(Cr, L) + np.asarray(bk, np.float32).reshape(Cr, 1)
    xw = np.zeros((128, 904), np.float16)
    xw[:, 0:32] = np.asarray(Wq, np.float32)[0:128]
    xw[:, 32:64] = np.asarray(Wq, np.float32)[128:256]
    xw[:, 64:96] = np.asarray(Wk, np.float32)[0:128]
    xw[:, 96:128] = np.asarray(Wk, np.float32)[128:256]
    xw[:, 128:384] = np.asarray(Wv, np.float32)[0:128]
    xw[:, 384:640] = np.asarray(Wv, np.float32)[128:256]
    xw[:, 640:896] = np.asarray(bv, np.float32).reshape(1, C)
    bqa = np.asarray(bq, np.float32).reshape(Cr, 1)

    in_maps = []
    for i in range(NCORES):
        b, h = divmod(i, 2)
        xT = x[b].T.astype(np.float16)
        if h == 1:
            xT = np.concatenate([xT[:, MH:], xT[:, :MH]], axis=1)
        xtc = np.ascontiguousarray(
            np.concatenate([xT[0:128], xT[128:256]], axis=1))
        posh = np.ascontiguousarray(posf[:, h * MH:(h + 1) * MH])
        in_maps.append({"xt": xtc, "xw": xw, "pos": posh, "bq": bqa})
    return in_maps


def _combine(results):
    out = np.empty((B, L, C), dtype=np.float32)
    for b in range(B):
        o0 = results[2 * b]["outT"].astype(np.float32).sum(axis=1)
        o1 = results[2 * b + 1]["outT"].astype(np.float32).sum(axis=1)
        o1 = np.concatenate([o1[:, MH:], o1[:, :MH]], axis=1)
        out[b] = (o0 + o1).T
    return out


def kernel(**inputs):
    from concourse.bass_utils import run_bass_kernel_spmd

    nc = _get_nc("full")
    in_maps = _prep_core_inputs(**inputs)
    res = run_bass_kernel_spmd(nc, in_maps, core_ids=list(range(NCORES)))
    return _combine(res.results)


# revision 3
# speedup vs baseline: 1.0207x; 1.0207x over previous
"""Trainium2 Bass kernel v2 for nn_Att_61881888801149 (sparse_attention).

Math per batch b (softmax over the QUERY axis l):
    out[l, c] = sum_m E[l, m] * v[m, c] / colsum[m],  E = exp(S - 11.5)
    S = q (k^T + pos)

Sharding: 8 cores = 4 batches x 2 key-halves (M = 2048 keys per core).
Host rotates xT columns per-core so each core's m-half is always columns
0:2048; outputs are un-rotated and pair-summed on the host.

v2 schedule (vs v1, 114.4us -> target ~96us): the v1 timeline was
  [ACT-bound exp stream 4..88us, PE ~45% idle] then [PE-only phase-2
  tail 88..110us]. Here phase-2 runs INSIDE the exp stream:
  - out accumulation is split into NG m-slabs; each slab's partial out
    tile goes PSUM -> DRAM directly (no on-chip add chain); the host sums
    the slabs.  This keeps DVE off the critical path entirely and lets
    slabs be small (fine-grained PE feed as each E[mb] lands).
  - PSUM: 2x[128,1024] ST chunks (ACT-paced) + 4x[128,512] rotating
    slots shared by projection psums (early) and slab accumulators.
  - q/v bias matmuls removed: folded into the DVE psum->SBUF copies
    (bq rides in pos's last column; bv is pre-broadcast in xw rows).
  - colsum reduce + reciprocal on DVE; vbw scaling on the idle Pool.
"""

import sys

for _p in ("/opt/trn_rl_repo", "/root/.axon_site/_ro/trn_rl_repo"):
    if _p not in sys.path:
        sys.path.append(_p)

import numpy as np

B, L, C, Cr = 4, 4096, 256, 32
MH = L // 2
NCORES = 8

_CACHE = {}

# phase-2 m-slabs: per out-tile staggered segment boundaries so chain
# ready-times spread uniformly across the exp stream (NMB=16 m-blocks).
# 4 segments per tile -> outT carries 4 partial slabs summed on the host.
NG = 4


def tile_segs(t):
    s1 = 2 + (t * 5) // 16          # 2..6
    s2 = 6 + (t * 8) // 16          # 6..13
    s3 = 14 + (t * 2) // 16         # 14..15
    return [0, s1, s2, s3, 16]


def build_nc(L=L, C=C, Cr=Cr, M=MH):
    import concourse.bass as bass
    import concourse.tile as tile
    from concourse import mybir
    from concourse.tile_rust import add_dep_helper

    FP32 = mybir.dt.float32
    FP16 = mybir.dt.float16
    Exp = mybir.ActivationFunctionType.Exp
    EXP_OFF = 11.5

    assert C == 256 and Cr == 32
    NMB = M // 128          # 16 m-blocks per core
    NLG = L // 512          # 8 l-groups
    SG = 1024               # ACT exp chunk width
    NSG = L // SG           # 4 stats groups per m-block
    NST = NSG + 1           # +1: mb0's first chunk is split in two
    NCH = C // 128          # 2 output channel halves
    NT = NLG * NCH          # 16 out tiles per slab

    # xw (fp16) columns: wq0 0:32 | wq1 32:64 | wk0 64:96 | wk1 96:128 |
    # wv0 128:384 | wv1 384:640 | bv broadcast (all rows) 640:896
    nc = bass.Bass()
    xt_d = nc.dram_tensor("xt", [128, 2 * L], FP16, kind="ExternalInput")
    xw_d = nc.dram_tensor("xw", [128, 904], FP16, kind="ExternalInput")
    pos_d = nc.dram_tensor("pos", [Cr, M], FP32, kind="ExternalInput")
    bq_d = nc.dram_tensor("bq", [Cr, 1], FP32, kind="ExternalInput")
    outT_d = nc.dram_tensor("outT", [C, NG, L], FP16, kind="ExternalOutput")

    with tile.TileContext(nc) as tc:
        with (
            tc.tile_pool(name="persist", bufs=1) as persist,
            tc.tile_pool(name="psum", bufs=1, space="PSUM") as psum,
        ):
            qT = persist.tile([Cr, L], FP16)
            kpT = persist.tile([Cr, M], FP16)
            vb = persist.tile([128, NMB, C], FP16)
            vbw = persist.tile([128, NMB, C], FP16)
            stats = persist.tile([128, NMB, NST], FP32)
            nc.vector.memset(stats[:, :, NSG:], 0.0)
            colsum = persist.tile([128, NMB], FP32)
            wrec = persist.tile([128, NMB], FP32)
            expoff = persist.tile([128, 1], FP32)
            nc.vector.memset(expoff[:], -EXP_OFF)
            # carrier tile: hoists PSUM-slot WAR waits onto a PE Ldweights
            # so the slot-opening Matmult keeps 1 sem wait (walrus budget)
            wdum = persist.tile([1, 1], FP16)
            nc.vector.memset(wdum[:], 0.0)
            # warm the ACT exp table before the stream
            exw = persist.tile([1, 1], FP32)
            nc.scalar.activation(exw[:], expoff[0:1, 0:1], Exp)

            def carrier(dep):
                if dep is None:
                    return None
                c = nc.tensor.ldweights(wdum[:])
                add_dep_helper(c.ins, dep.ins, sync=True,
                               reason="psum slot WAR carrier")
                return c

            def anchor(mm, c):
                if c is not None:
                    add_dep_helper(mm.ins, c.ins, sync=False,
                                   reason="carrier anchor")
                return mm

            with tc.tile_pool(name="epool", bufs=1) as epool:
                E = epool.tile([128, NMB, L], FP16)

                with tc.tile_pool(name="prolog", bufs=1) as pp:
                    bqc = pp.tile([Cr, 1], FP32)
                    xw = pp.tile([128, 904], FP16)
                    pos = pp.tile([Cr, M], FP32)
                    xt = pp.tile([128, 2, L], FP16)
                    # sync queue: xt half 0 (first 512 lands first).
                    # gpsimd queue: weights/bias/pos interleaved with xt
                    # half 1 so everything the first chunks need lands early.
                    # No DMAs on the ACT queue (they'd block the exp stream).
                    spans = [(0, 256), (256, 512), (512, 1024)] + [
                        (j * 1024, (j + 1) * 1024) for j in range(1, L // 1024)]
                    for a, b in spans:
                        nc.sync.dma_start(xt[:, 0, a:b],
                                          xt_d[:, a:b])
                    nc.gpsimd.dma_start(xw[:], xw_d[:])
                    nc.gpsimd.dma_start(xt[:, 1, 0:512], xt_d[:, L:L + 512])
                    nc.gpsimd.dma_start(bqc[:], bq_d[:])
                    nc.gpsimd.dma_start(xt[:, 1, 512:1024],
                                        xt_d[:, L + 512:L + 1024])
                    nc.gpsimd.dma_start(pos[:, 0:512], pos_d[:, 0:512])
                    for j in range(1, L // 1024):
                        nc.gpsimd.dma_start(
                            xt[:, 1, j * 1024:(j + 1) * 1024],
                            xt_d[:, L + j * 1024:L + (j + 1) * 1024])
                    nc.gpsimd.dma_start(pos[:, 512:M], pos_d[:, 512:M])

                    wq0, wq1 = xw[:, 0:32], xw[:, 32:64]
                    wk0, wk1 = xw[:, 64:96], xw[:, 96:128]
                    wv0, wv1 = xw[:, 128:384], xw[:, 384:640]
                    bvb = xw[:, 640:896]

                    # rotating 4-slot PSUM pool: proj psums early, slab
                    # accumulators after
                    hist_oa = [None, None, None, None]
                    hist_st = [None, None]
                    kidx = [0]

                    def oa_tile(name):
                        cr_ = carrier(hist_oa[kidx[0] % 4])
                        t = psum.tile([128, 512], FP32, tag="oa", bufs=4,
                                      name=name)
                        return t, cr_

                    def oa_done(reader):
                        hist_oa[kidx[0] % 4] = reader
                        kidx[0] += 1

                    qdone, kdone, vdone = set(), set(), set()

                    def need_q(j):
                        if j in qdone:
                            return
                        qdone.add(j)
                        sl = slice(j * 512, (j + 1) * 512)
                        psq_t, cr_ = oa_tile(f"psq_{j}")
                        psq = psq_t[0:Cr, 0:512]
                        anchor(nc.tensor.matmul(psq[:], wq0, xt[:, 0, sl],
                                                start=True, stop=False), cr_)
                        nc.tensor.matmul(psq[:], wq1, xt[:, 1, sl],
                                         start=False, stop=True)
                        oa_done(nc.vector.tensor_scalar_add(
                            qT[:, sl], psq[:], bqc))

                    def need_k(j):
                        if j in kdone:
                            return
                        kdone.add(j)
                        sl = slice(j * 512, (j + 1) * 512)
                        psk_t, cr_ = oa_tile(f"psk_{j}")
                        psk = psk_t[0:Cr, 0:512]
                        anchor(nc.tensor.matmul(psk[:], wk0, xt[:, 0, sl],
                                                start=True, stop=False), cr_)
                        nc.tensor.matmul(psk[:], wk1, xt[:, 1, sl],
                                         start=False, stop=True)
                        oa_done(nc.vector.tensor_add(
                            kpT[:, sl], psk[:], pos[:, sl]))

                    def need_v(mb):
                        if mb in vdone:
                            return
                        vdone.add(mb)
                        msl = slice(mb * 128, (mb + 1) * 128)
                        psv_t, cr_ = oa_tile(f"psv_{mb}")
                        psv = psv_t[:, 0:C]
                        anchor(nc.tensor.matmul(psv[:], xt[:, 0, msl], wv0,
                                                start=True, stop=False), cr_)
                        nc.tensor.matmul(psv[:], xt[:, 1, msl], wv1,
                                         start=False, stop=True)
                        oa_done(nc.vector.tensor_add(vb[:, mb, :], psv[:],
                                                     bvb))

                    def stats_tail(mb):
                        nst = NST if mb == 0 else NSG
                        nc.vector.reduce_sum(colsum[:, mb:mb + 1],
                                             stats[:, mb, 0:nst],
                                             axis=mybir.AxisListType.X)
                        nc.vector.reciprocal(wrec[:, mb:mb + 1],
                                             colsum[:, mb:mb + 1])
                        nc.vector.tensor_scalar_mul(
                            vbw[:, mb, :], vb[:, mb, :], wrec[:, mb:mb + 1])

                    stc = [0]

                    def st_chunk(mb, a, b, si_):
                        kp_sl = kpT[:, mb * 128:(mb + 1) * 128]
                        idx = stc[0]
                        stc[0] += 1
                        cr_ = carrier(hist_st[idx % 2])
                        ps = psum.tile([128, SG], FP32, tag="st", bufs=2)
                        for j, c0 in enumerate(range(a, b, 512)):
                            mm = nc.tensor.matmul(
                                ps[:, j * 512:j * 512 + 512],
                                kp_sl, qT[:, c0:c0 + 512],
                                start=True, stop=True)
                            if j == 0:
                                anchor(mm, cr_)
                        last_exp = nc.scalar.activation(
                            E[:, mb, a:b], ps[:, 0:b - a], Exp,
                            bias=expoff[:],
                            accum_out=stats[:, mb, si_:si_ + 1])
                        hist_st[idx % 2] = last_exp

                    # ---- phase-2 slab chains ----
                    # chain (si, t): accumulate the si-th mb segment of out
                    # tile t=(lg,ch) in a PSUM slot (depth-first), flush via
                    # an engine copy to fp16 staging, then DMA the partial
                    # slab to DRAM (the host sums the NG slabs).
                    dmaq = [nc.sync, nc.sync, nc.gpsimd]
                    Copy = mybir.ActivationFunctionType.Copy
                    flip = [0]
                    dmn = [0]

                    def chain(si, t):
                        segs = tile_segs(t)
                        m0, m1 = segs[si], segs[si + 1]
                        lg, ch = t // NCH, t % NCH
                        lsl = slice(lg * 512, (lg + 1) * 512)
                        poa, cr_ = oa_tile(f"poa_{si}_{t}")
                        for mb in range(m0, m1):
                            mm = nc.tensor.matmul(
                                poa[:],
                                vbw[:, mb, ch * 128:(ch + 1) * 128],
                                E[:, mb, lsl],
                                start=(mb == m0), stop=(mb == m1 - 1))
                            if mb == m0:
                                anchor(mm, cr_)
                        stg = stage.tile([128, 512], FP16, tag="stg")
                        if si == NG - 1 and flip[0] % 2 == 0:
                            rd = nc.scalar.activation(stg[:], poa[:], Copy)
                        else:
                            rd = nc.vector.tensor_copy(stg[:], poa[:])
                        if si == NG - 1:
                            flip[0] += 1
                        oa_done(rd)
                        if si == NG - 1:
                            dq = [nc.gpsimd, nc.sync][dmn[0] % 2]
                        else:
                            dq = dmaq[dmn[0] % 3]
                        dq.dma_start(
                            outT_d[ch * 128:(ch + 1) * 128, si, lsl],
                            stg[:])
                        dmn[0] += 1

                    # static emission schedule keyed by exp-chunk index:
                    # chain (si,t) is ready once E[m1-1] lands (chunk
                    # 4*m1-1); emit with +2 skew (PE runs ~2 chunks ahead),
                    # capped at one chain-open per chunk slot.
                    todo = []
                    for t in range(NT):
                        segs = tile_segs(t)
                        for si in range(NG):
                            todo.append((4 * segs[si + 1] + 3, si, t))
                    todo.sort()
                    events = {}
                    nfree = 0
                    for rci, si, t in todo:
                        ci = max(rci, nfree)
                        nfree = ci + 1
                        events.setdefault(ci, []).append((si, t))

                    def run_events(ci):
                        for si, t in events.pop(ci, []):
                            chain(si, t)

                    with tc.tile_pool(name="stage", bufs=6) as stage:
                        # all 28 projection psums acquire slots before any
                        # chain opens (slot rotation deadlock avoidance)
                        need_q(0)
                        need_q(1)
                        need_k(0)
                        cic = [0]

                        def after_chunk(mb, g):
                            if mb == 0:
                                need_v(g)
                                if g == 3:
                                    need_k(1)
                            elif mb in (1, 2, 3):
                                need_v(4 * mb + g)
                                if g == 3 and mb < 3:
                                    need_k(mb + 1)
                            if g == 0 and mb > 0:
                                stats_tail(mb - 1)
                            run_events(cic[0])
                            cic[0] += 1

                        # mb0: five chunks (512,512,1024x3) so the exp
                        # stream starts as soon as q g0 alone is ready
                        mb0_spans = [(0, 512, 0), (512, 1024, 4),
                                     (1024, 2048, 1), (2048, 3072, 2),
                                     (3072, 4096, 3)]
                        qpre = {0: (2, 3), 1: (4, 5), 2: (6, 7)}
                        for i, (a, b, si_) in enumerate(mb0_spans):
                            st_chunk(0, a, b, si_)
                            for qg in qpre.get(i, ()):
                                need_q(qg)
                            after_chunk(0, min(i, 3))
                        for mb in range(1, NMB):
                            for g in range(NSG):
                                st_chunk(mb, g * SG, (g + 1) * SG, g)
                                after_chunk(mb, g)
                        stats_tail(NMB - 1)
                        for ci in sorted(events):
                            for si, t in events[ci]:
                                chain(si, t)

    return nc


def _fixup_waits(nc):
    """Hoist excess semaphore waits (>1 per TPB instruction) into
    standalone EventSemaphore instructions on the same engine."""
    from concourse import mybir

    budget_by_type = {}
    n = 0
    for fn in nc.m.functions:
        for blk in fn.blocks:
            insts = blk.instructions
            i = 0
            while i < len(insts):
                inst = insts[i]
                si = getattr(inst, "sync_info", None)
                if si is None:
                    i += 1
                    continue
                waits = list(si.on_wait)
                budget = budget_by_type.get(type(inst).__name__, 1)
                if len(waits) <= budget:
                    i += 1
                    continue
                extra, keep = waits[:-budget], waits[-budget:]
                for w in extra:
                    es = mybir.InstEventSemaphore(
                        name=f"es_waitfix_{n}", ins=[], outs=[])
                    n += 1
                    es.engine = inst.engine
                    es.sync_info = mybir.SyncInfo(on_wait=[w], on_update=[])
                    insts.insert(i, es)
                    i += 1
                inst.sync_info = mybir.SyncInfo(
                    on_wait=keep, on_update=list(si.on_update))
                i += 1


def _build_and_fix(**kw):
    nc = build_nc(**kw)
    _fixup_waits(nc)
    return nc


def _get_nc(key, **kw):
    if key not in _CACHE:
        _CACHE[key] = _build_and_fix(**kw)
    return _CACHE[key]


def _prep_core_inputs(x, rel_h, rel_w, Wq, bq, Wk, bk, Wv, bv):
    x = np.asarray(x, dtype=np.float32)
    posf = (np.asarray(rel_h, np.float32) + np.asarray(rel_w, np.float32))
    posf = posf.reshape(Cr, L) + np.asarray(bk, np.float32).reshape(Cr, 1)
    xw = np.zeros((128, 904), np.float16)
    xw[:, 0:32] = np.asarray(Wq, np.float32)[0:128]
    xw[:, 32:64] = np.asarray(Wq, np.float32)[128:256]
    xw[:, 64:96] = np.asarray(Wk, np.float32)[0:128]
    xw[:, 96:128] = np.asarray(Wk, np.float32)[128:256]
    xw[:, 128:384] = np.asarray(Wv, np.float32)[0:128]
    xw[:, 384:640] = np.asarray(Wv, np.float32)[128:256]
    xw[:, 640:896] = np.asarray(bv, np.float32).reshape(1, C)
    bqa = np.asarray(bq, np.float32).reshape(Cr, 1)

    in_maps = []
    for i in range(NCORES):
        b, h = divmod(i, 2)
        xT = x[b].T.astype(np.float16)
        if h == 1:
            xT = np.concatenate([xT[:, MH:], xT[:, :MH]], axis=1)
        xtc = np.ascontiguousarray(
            np.concatenate([xT[0:128], xT[128:256]], axis=1))
        posh = np.ascontiguousarray(posf[:, h * MH:(h + 1) * MH])
        in_maps.append({"xt": xtc, "xw": xw, "pos": posh, "bq": bqa})
    return in_maps


def _combine(results):
    out = np.empty((B, L, C), dtype=np.float32)
    for b in range(B):
        o0 = results[2 * b]["outT"].astype(np.float32).sum(axis=1)
        o1 = results[2 * b + 1]["outT"].astype(np.float32).sum(axis=1)
        o1 = np.concatenate([o1[:, MH:], o1[:, :MH]], axis=1)
        out[b] = (o0 + o1).T
    return out


def kernel(**inputs):
    from concourse.bass_utils import run_bass_kernel_spmd

    nc = _get_nc("full")
    in_maps = _prep_core_inputs(**inputs)
    res = run_bass_kernel_spmd(nc, in_maps, core_ids=list(range(NCORES)))
    return _combine(res.results)
